# revision 6
# baseline (speedup 1.0000x reference)
"""GCN (2-layer GCNConv + log_softmax) on 8 Trainium2 NeuronCores.

Strategy (graph/data parallel, per sharding hint):
  - Nodes row-sharded across 8 cores (dst shards of P=N/8).
  - norm factorizes: norm(e) = dinv[src]*dinv[dst].  Tables store
    Z' = dinv * (x @ W1); output scaled by dinv[dst].  Self-loop becomes an
    ordinary slot gathering the dst's own table row.
  - Per layer: every core gathers its dst-shard's edge messages from a
    replicated DRAM table via indirect DMA (one 64B descriptor per edge
    slot), host-arranged so each dst's slots are contiguous and padded to a
    degree-bucket multiple of 8 -> segment-sum is an affine tensor_reduce.
  - Tables are built by AllGather of per-core shards (layer2 table lives in
    host-chosen "bucket order"; gather indices absorb the permutation).
  - logits = (A_hat @ H1) @ W2 + b2 (aggregation commutes with W2), then a
    batched log_softmax over C=40 on-chip.

Host-side work is limited to graph preprocessing (sharding, degree counts,
index/permutation tables) and input layout; all feature math runs on device.
"""

import os
import sys

for _p in ("/opt/trn_rl_repo", "/opt/pypackages"):
    if _p not in sys.path:
        sys.path.insert(0, _p)

import numpy as np

from concourse import bacc, bass, tile, mybir
from concourse import bass_utils

F32 = mybir.dt.float32
I32 = mybir.dt.int32
AF = mybir.ActivationFunctionType
ALU = mybir.AluOpType

NC = 8
CHUNK_SLOTS = 512  # max gather slots per partition per indirect-DMA chunk

_last_result = {}


# ---------------------------------------------------------------------------
# Host-side graph preprocessing
# ---------------------------------------------------------------------------

def _make_plan(src, dst, N):
    """Build per-core slot/bucket layout. Returns a dict of numpy arrays and
    layout constants shared by the device program (uniform across cores)."""
    P = N // NC
    E = src.shape[0]

    deg = np.bincount(dst, minlength=N).astype(np.int64) + 1  # incl self-loop
    dinv = (1.0 / np.sqrt(deg.astype(np.float64))).astype(np.float32)

    core_of = (dst // P).astype(np.int64)
    dloc_all = dst - core_of * P

    # slots per dst: self-loop + in-edges, rounded up to multiple of 8
    slots_of = ((deg + 7) // 8) * 8  # deg already includes +1

    # global bucket set (uniform program across cores)
    Ks = np.unique(slots_of)
    # tiles per bucket = max over cores of ceil(count/128)
    T_of_K = {}
    counts = np.zeros((NC, len(Ks)), dtype=np.int64)
    kidx_of_slots = {int(k): i for i, k in enumerate(Ks)}
    slots_loc = slots_of.reshape(NC, P)  # node n = c*P + l
    for c in range(NC):
        ks, cnt = np.unique(slots_loc[c], return_counts=True)
        for k, n in zip(ks, cnt):
            counts[c, kidx_of_slots[int(k)]] = n
    for i, k in enumerate(Ks):
        T_of_K[int(k)] = int((counts[:, i].max() + 127) // 128)

    # force even total tile count (for paired 32-wide DVE transposes)
    T_tot = sum(T_of_K.values())
    if T_tot % 2 == 1:
        T_of_K[int(Ks[0])] += 1
        T_tot += 1

    # bucket layout: tile base and idx free-offset per bucket (ascending K)
    tile_base = {}
    off_base = {}
    tb, ob = 0, 0
    for k in Ks:
        k = int(k)
        tile_base[k] = tb
        off_base[k] = ob
        tb += T_of_K[k]
        ob += T_of_K[k] * k
    SPP = ob            # idx slots per partition
    P_OUT = T_tot * 128  # padded output rows per core

    SENT1 = N            # zero row in table1
    SENT2 = NC * P_OUT   # zero row in table2

    # ---- per-node positions ------------------------------------------------
    # within each core: order dsts by (K bucket asc, node id asc)
    # q(node) = tile*128 + p   (tile counted globally, bucket-major)
    q_of = np.empty(N, dtype=np.int64)
    dinvd = np.zeros((NC, 128, T_tot), dtype=np.float32)
    out_rows = np.full((NC, P_OUT), -1, dtype=np.int64)  # q -> global node
    for c in range(NC):
        sl = slots_loc[c]
        order = np.lexsort((np.arange(P), sl))  # by K, then node id
        kk = sl[order]
        # position within bucket
        pos_in_bucket = np.zeros(P, dtype=np.int64)
        start = 0
        qs = np.empty(P, dtype=np.int64)
        for k in np.unique(kk):
            k = int(k)
            idxs = np.where(kk == k)[0]
            rel = np.arange(len(idxs))
            qs[idxs] = tile_base[k] * 128 + rel
            start += len(idxs)
        nodes = c * P + order
        q_of[nodes] = qs
        out_rows[c, qs] = nodes
        dinvd[c, qs % 128, qs // 128] = dinv[nodes]

    # table2 row of node n: core(n)*P_OUT + q(n)
    pos2 = (np.arange(N) // P) * P_OUT + q_of

    # ---- per-edge slot positions -------------------------------------------
    # edge e (dst d): slot col = off_base[K(d)] + (q(d)//128)*K + 1 + rank(e)
    # partition = q(d) % 128 ; rank = index among edges with same dst
    idx1 = np.full((NC, 128, SPP), SENT1, dtype=np.int32)
    idx2 = np.full((NC, 128, SPP), SENT2, dtype=np.int32)

    # self-loop slots (col offset 0) for every real dst
    for c in range(NC):
        nodes = np.arange(c * P, (c + 1) * P)
        q = q_of[nodes]
        k = slots_of[nodes]
        koff = np.array([off_base[int(x)] for x in np.unique(k)])  # placeholder
        # vector off_base lookup
        offv = np.zeros(len(nodes), dtype=np.int64)
        tbv = np.zeros(len(nodes), dtype=np.int64)
        for kk_ in np.unique(k):
            m = k == kk_
            offv[m] = off_base[int(kk_)]
            tbv[m] = tile_base[int(kk_)]
        col = offv + (q // 128 - tbv) * k
        idx1[c, q % 128, col] = nodes.astype(np.int32)
        idx2[c, q % 128, col] = pos2[nodes].astype(np.int32)

    # edge slots
    eorder = np.argsort(dst, kind="stable")
    s_sorted = src[eorder]
    d_sorted = dst[eorder]
    starts = np.searchsorted(d_sorted, np.arange(N))
    rank = np.arange(E) - starts[d_sorted]

    q_e = q_of[d_sorted]
    k_e = slots_of[d_sorted]
    offv = np.zeros(E, dtype=np.int64)
    tbv = np.zeros(E, dtype=np.int64)
    for kk_ in np.unique(k_e):
        m = k_e == kk_
        offv[m] = off_base[int(kk_)]
        tbv[m] = tile_base[int(kk_)]
    col_e = offv + (q_e // 128 - tbv) * k_e + 1 + rank
    c_e = d_sorted // P
    p_e = q_e % 128
    idx1[c_e, p_e, col_e] = s_sorted.astype(np.int32)
    idx2[c_e, p_e, col_e] = pos2[s_sorted].astype(np.int32)

    # ---- gather chunks ------------------------------------------------------
    chunks = []  # (idx_off, spp, tile0, ntiles, K)
    for k in Ks:
        k = int(k)
        T = T_of_K[k]
        per = max(1, CHUNK_SLOTS // k)
        t0 = 0
        while t0 < T:
            ct = min(per, T - t0)
            chunks.append(
                (off_base[k] + t0 * k, ct * k, tile_base[k] + t0, ct, k)
            )
            t0 += ct

    return dict(
        P=P, dinv=dinv, T_tot=T_tot, SPP=SPP, P_OUT=P_OUT,
        SENT1=SENT1, SENT2=SENT2, idx1=idx1, idx2=idx2,
        dinvd=dinvd, out_rows=out_rows, chunks=chunks,
    )


# ---------------------------------------------------------------------------
# Device program
# ---------------------------------------------------------------------------

def _ap(t_ap, offset, dims):
    return bass.AP(t_ap.tensor, t_ap.offset + offset, [list(t_ap.ap[0])] + dims)


def _dap(t_ap, offset, dims):
    return bass.AP(t_ap.tensor, t_ap.offset + offset, dims)


def _build_program(N, F, HID, C, plan):
    P = plan["P"]
    T_tot = plan["T_tot"]
    SPP = plan["SPP"]
    P_OUT = plan["P_OUT"]
    chunks = plan["chunks"]
    NT_X = (P + 127) // 128
    PN = NT_X * 128
    KC = F // 128  # contraction chunks for x@W1

    nc = bacc.Bacc(None, target_bir_lowering=False, debug=False, num_devices=NC)

    xT_d = nc.dram_tensor("xT", [F, PN], F32, kind="ExternalInput")
    w1_d = nc.dram_tensor("W1", [F, HID], F32, kind="ExternalInput")
    b1_d = nc.dram_tensor("b1r", [128, HID], F32, kind="ExternalInput")
    GT = 8  # tiles per logits matmul group (block-diag W2)
    w2_d = nc.dram_tensor("W2blk", [GT * HID, GT * C], F32,
                          kind="ExternalInput")
    b2_d = nc.dram_tensor("b2r", [128, C], F32, kind="ExternalInput")
    dinvx_d = nc.dram_tensor("dinvx", [128, NT_X], F32, kind="ExternalInput")
    dinvd_d = nc.dram_tensor("dinvd", [128, T_tot], F32, kind="ExternalInput")
    idx1_d = nc.dram_tensor("idx1", [128, SPP], I32, kind="ExternalInput")
    idx2_d = nc.dram_tensor("idx2", [128, SPP], I32, kind="ExternalInput")
    out_d = nc.dram_tensor("out", [128, T_tot * C], F32, kind="ExternalOutput")
    debug = bool(int(os.environ.get("GCN_DEBUG", "0")))
    if debug:
        dbg_z = nc.dram_tensor("dbg_z", [PN, HID], F32, kind="ExternalOutput")
        dbg_t1 = nc.dram_tensor("dbg_t1", [N + 8, HID], F32,
                                kind="ExternalOutput")
        dbg_a1 = nc.dram_tensor("dbg_a1", [128, T_tot * HID], F32,
                                kind="ExternalOutput")
        dbg_h = nc.dram_tensor("dbg_h", [P_OUT, HID], F32,
                               kind="ExternalOutput")
        dbg_t2 = nc.dram_tensor("dbg_t2", [NC * P_OUT + 8, HID], F32,
                                kind="ExternalOutput")
        dbg_a2 = nc.dram_tensor("dbg_a2", [128, T_tot * HID], F32,
                                kind="ExternalOutput")
        dbg_mg = nc.dram_tensor("dbg_mg", [128, SPP * HID], F32,
                                kind="ExternalOutput")

    with tile.TileContext(nc) as tc:
        with (
            tc.tile_pool(name="const", bufs=1) as cp,
            tc.tile_pool(name="dram", bufs=1, space="DRAM") as dp,
            tc.tile_pool(name="xt", bufs=4) as xtp,
            tc.tile_pool(name="zpsum", bufs=2, space="PSUM") as zpp,
            tc.tile_pool(name="zsb", bufs=2) as zsp,
            tc.tile_pool(name="idx", bufs=2) as ixp,
            tc.tile_pool(name="msgs", bufs=2) as mgp,
            tc.tile_pool(name="agg", bufs=1) as agp,
            tc.tile_pool(name="lpsum", bufs=2, space="PSUM") as lpp,
        ):
            # ---- constants ----
            w1 = []
            for kc in range(KC):
                t = cp.tile([128, HID], F32, tag=f"w1_{kc}")
                nc.sync.dma_start(out=t[:], in_=w1_d[kc * 128:(kc + 1) * 128, :])
                w1.append(t)
            b1r = cp.tile([128, HID], F32, tag="b1r")
            nc.sync.dma_start(out=b1r[:], in_=b1_d[:, :])
            w2 = cp.tile([GT * HID, GT * C], F32, tag="w2")
            nc.sync.dma_start(out=w2[:], in_=w2_d[:, :])
            b2r = cp.tile([128, C], F32, tag="b2r")
            nc.sync.dma_start(out=b2r[:], in_=b2_d[:, :])
            dinvx = cp.tile([128, NT_X], F32, tag="dinvx")
            nc.sync.dma_start(out=dinvx[:], in_=dinvx_d[:, :])
            dinvd = cp.tile([128, T_tot], F32, tag="dinvd")
            nc.sync.dma_start(out=dinvd[:], in_=dinvd_d[:, :])
            zrow = cp.tile([8, HID], F32, tag="zrow")
            nc.vector.memset(zrow[:], 0.0)

            # ---- DRAM scratch ----
            z_shard = dp.tile([PN, HID], F32, tag="z_shard")
            table1 = dp.tile([N + 8, HID], F32, tag="table1")
            hsh = dp.tile([P_OUT, HID], F32, tag="hsh")
            table2 = dp.tile([NC * P_OUT + 8, HID], F32, tag="table2")

            nc.sync.dma_start(out=table1[N:N + 8, :], in_=zrow[:])
            nc.sync.dma_start(
                out=table2[NC * P_OUT:NC * P_OUT + 8, :], in_=zrow[:]
            )

            # ---- Z' = dinv * (x @ W1) for own shard ----
            GZ = 8  # node tiles per PSUM tile
            for g0 in range(0, NT_X, GZ):
                gn = min(GZ, NT_X - g0)
                zp = zpp.tile([128, GZ * HID], F32, tag="zp")
                for ti in range(gn):
                    t = g0 + ti
                    for kc in range(KC):
                        xa = xtp.tile([128, 128], F32, tag="xa")
                        nc.sync.dma_start(
                            out=xa[:],
                            in_=xT_d[kc * 128:(kc + 1) * 128,
                                     t * 128:(t + 1) * 128],
                        )
                        nc.tensor.matmul(
                            out=zp[:, ti * HID:(ti + 1) * HID],
                            lhsT=xa[:],
                            rhs=w1[kc][:],
                            start=(kc == 0),
                            stop=(kc == KC - 1),
                        )
                zs = zsp.tile([128, GZ * HID], F32, tag="zs")
                nc.vector.tensor_tensor(
                    out=_ap(zs[:], 0, [[HID, gn], [1, HID]]),
                    in0=_ap(zp[:], 0, [[HID, gn], [1, HID]]),
                    in1=_ap(dinvx[:], g0, [[1, gn], [0, HID]]),
                    op=ALU.mult,
                )
                # store rows t*128+p for t in [g0, g0+gn)
                nc.sync.dma_start(
                    out=_dap(z_shard[:, :], g0 * 128 * HID,
                             [[HID, 128], [128 * HID, gn], [1, HID]]),
                    in_=_ap(zs[:], 0, [[HID, gn], [1, HID]]),
                )

            # ---- AllGather Z' -> table1 ----
            nc.gpsimd.collective_compute(
                "AllGather",
                ALU.bypass,
                replica_groups=[list(range(NC))],
                ins=[z_shard[0:P, :]],
                outs=[table1[0:NC * P, :]],
            )

            agg = agp.tile([128, T_tot * HID], F32, tag="agg")
            h1p = agp.tile([128, T_tot * HID], F32, tag="h1p")

            def gather_layer(idx_dram, table_t, nrows, dump=False):
                for (ioff, spp, tile0, ntiles, K) in chunks:
                    it = ixp.tile([128, CHUNK_SLOTS], I32, tag="it")
                    nc.sync.dma_start(
                        out=it[:, 0:spp], in_=idx_dram[:, ioff:ioff + spp]
                    )
                    mg = mgp.tile([128, CHUNK_SLOTS * HID], F32, tag="mg")
                    # HW consumes ONE offset per partition per contiguous
                    # out-run, so issue one call per slot column.
                    for s in range(spp):
                        nc.gpsimd.indirect_dma_start(
                            out=mg[:, s * HID:(s + 1) * HID],
                            out_offset=None,
                            in_=table_t[:, :],
                            in_offset=bass.IndirectOffsetOnAxis(
                                ap=it[:, s:s + 1], axis=0
                            ),
                        )
                    if dump:
                        nc.sync.dma_start(
                            out=dbg_mg[:, ioff * HID:(ioff + spp) * HID],
                            in_=mg[:, 0:spp * HID],
                        )
                    # segment-sum: [p, ntiles, HID, K] -> [p, ntiles, HID]
                    nc.vector.tensor_reduce(
                        out=_ap(agg[:], tile0 * HID, [[HID, ntiles], [1, HID]]),
                        in_=_ap(mg[:], 0,
                                [[K * HID, ntiles], [1, HID], [HID, K]]),
                        axis=mybir.AxisListType.X,
                        op=ALU.add,
                    )

            # ---- layer 1 ----
            gather_layer(idx1_d, table1, N + 8, dump=debug)
            if debug:
                nc.sync.dma_start(out=dbg_z[:, :], in_=z_shard[:, :])
                nc.sync.dma_start(out=dbg_t1[:, :], in_=table1[:, :])
                nc.sync.dma_start(out=dbg_a1[:, :], in_=agg[:])
            dinvd_b = _ap(dinvd[:], 0, [[1, T_tot], [0, HID]])
            b1_b = _ap(b1r[:], 0, [[0, T_tot], [1, HID]])
            a3 = _ap(agg[:], 0, [[HID, T_tot], [1, HID]])
            h3 = _ap(h1p[:], 0, [[HID, T_tot], [1, HID]])
            nc.vector.tensor_tensor(out=h3, in0=a3, in1=dinvd_b, op=ALU.mult)
            nc.vector.tensor_tensor(out=h3, in0=h3, in1=b1_b, op=ALU.add)
            nc.scalar.activation(out=h1p[:], in_=h1p[:], func=AF.Relu)
            nc.vector.tensor_tensor(out=h3, in0=h3, in1=dinvd_b, op=ALU.mult)
            # store bucket-ordered rows q = t*128 + p
            nc.sync.dma_start(
                out=_dap(hsh[:, :], 0,
                         [[HID, 128], [128 * HID, T_tot], [1, HID]]),
                in_=h3,
            )

            # ---- AllGather H1' -> table2 ----
            nc.gpsimd.collective_compute(
                "AllGather",
                ALU.bypass,
                replica_groups=[list(range(NC))],
                ins=[hsh[:, :]],
                outs=[table2[0:NC * P_OUT, :]],
            )

            # ---- layer 2 ----
            gather_layer(idx2_d, table2, NC * P_OUT + 8)
            if debug:
                nc.sync.dma_start(out=dbg_h[:, :], in_=hsh[:, :])
                nc.sync.dma_start(out=dbg_t2[:, :], in_=table2[:, :])
                nc.sync.dma_start(out=dbg_a2[:, :], in_=agg[:])
            nc.vector.tensor_tensor(out=h3, in0=a3, in1=dinvd_b, op=ALU.mult)

            # ---- logits = agg2 @ W2 + b2 ; groups of 8 tiles ----
            # PE-transpose [128, 8*HID] -> PSUM [8*HID, 128], copy to SBUF,
            # then use [HID, 128] slices as lhsT.
            from concourse.masks import make_identity

            ident = cp.tile([128, 128], F32, tag="ident")
            make_identity(nc, ident[:])
            logits = agp.tile([128, T_tot * C], F32, tag="logits")
            for g0 in range(0, T_tot, GT):
                gn = min(GT, T_tot - g0)
                tps = lpp.tile([128, 128], F32, tag="tps")
                nc.tensor.transpose(
                    out=tps[0:gn * HID, :],
                    in_=h1p[:, g0 * HID:(g0 + gn) * HID],
                    identity=ident[:],
                )
                a2t = zsp.tile([128, 128], F32, tag="a2t")
                nc.vector.tensor_copy(
                    out=a2t[0:gn * HID, :], in_=tps[0:gn * HID, :]
                )
                lp = lpp.tile([128, GT * C], F32, tag="lp")
                nc.tensor.matmul(
                    out=lp[:, 0:gn * C],
                    lhsT=a2t[0:gn * HID, :],
                    rhs=w2[0:gn * HID, 0:gn * C],
                    start=True,
                    stop=True,
                )
                nc.vector.tensor_tensor(
                    out=_ap(logits[:], g0 * C, [[C, gn], [1, C]]),
                    in0=_ap(lp[:], 0, [[C, gn], [1, C]]),
                    in1=_ap(b2r[:], 0, [[0, gn], [1, C]]),
                    op=ALU.add,
                )

            # ---- log_softmax over C per row ----
            nmx = agp.tile([128, T_tot], F32, tag="nmx")
            sm = agp.tile([128, T_tot], F32, tag="sm")
            l3 = _ap(logits[:], 0, [[C, T_tot], [1, C]])
            nc.vector.tensor_reduce(
                out=nmx[:], in_=l3, axis=mybir.AxisListType.X,
                op=ALU.max, negate=True,
            )
            nc.vector.tensor_tensor(
                out=l3, in0=l3,
                in1=_ap(nmx[:], 0, [[1, T_tot], [0, C]]), op=ALU.add,
            )
            etile = mgp.tile([128, T_tot * C], F32, tag="mg")
            nc.scalar.activation(out=etile[:, 0:T_tot * C], in_=logits[:],
                                 func=AF.Exp)
            nc.vector.tensor_reduce(
                out=sm[:], in_=_ap(etile[:], 0, [[C, T_tot], [1, C]]),
                axis=mybir.AxisListType.X, op=ALU.add,
            )
            nc.scalar.activation(out=sm[:], in_=sm[:], func=AF.Ln)
            nc.vector.tensor_tensor(
                out=l3, in0=l3,
                in1=_ap(sm[:], 0, [[1, T_tot], [0, C]]), op=ALU.subtract,
            )
            nc.sync.dma_start(out=out_d[:, :], in_=logits[:])

    return nc


# ---------------------------------------------------------------------------
# Entry point
# ---------------------------------------------------------------------------

def kernel(x, edge_index, W1, b1, W2, b2):
    N, F = x.shape
    HID = W1.shape[1]
    C = W2.shape[1]
    P = N // NC
    src = np.asarray(edge_index[0], dtype=np.int64)
    dst = np.asarray(edge_index[1], dtype=np.int64)

    plan = _make_plan(src, dst, N)
    nc = _build_program(N, F, HID, C, plan)

    NT_X = (P + 127) // 128
    PN = NT_X * 128

    x = np.asarray(x, dtype=np.float32)
    W1 = np.ascontiguousarray(np.asarray(W1, dtype=np.float32))
    W2 = np.asarray(W2, dtype=np.float32)
    GT = 8
    W2blk = np.zeros((GT * HID, GT * C), dtype=np.float32)
    for g in range(GT):
        W2blk[g * HID:(g + 1) * HID, g * C:(g + 1) * C] = W2
    b1r = np.tile(np.asarray(b1, dtype=np.float32)[None, :], (128, 1))
    b2r = np.tile(np.asarray(b2, dtype=np.float32)[None, :], (128, 1))

    dinv = plan["dinv"]
    in_maps = []
    for c in range(NC):
        xT = np.zeros((F, PN), dtype=np.float32)
        xT[:, :P] = x[c * P:(c + 1) * P].T
        dinvx = np.zeros((128, NT_X), dtype=np.float32)
        sl = dinv[c * P:(c + 1) * P]
        pad = np.zeros(PN, dtype=np.float32)
        pad[:P] = sl
        dinvx[:, :] = pad.reshape(NT_X, 128).T
        in_maps.append({
            "xT": np.ascontiguousarray(xT),
            "W1": W1,
            "b1r": b1r,
            "W2blk": W2blk,
            "b2r": b2r,
            "dinvx": np.ascontiguousarray(dinvx),
            "dinvd": np.ascontiguousarray(plan["dinvd"][c]),
            "idx1": np.ascontiguousarray(plan["idx1"][c]),
            "idx2": np.ascontiguousarray(plan["idx2"][c]),
        })

    trace = bool(int(os.environ.get("GCN_TRACE", "0")))
    if int(os.environ.get("GCN_SIM", "0")):
        from concourse.bass_interp import MultiCoreSim

        sim = MultiCoreSim(nc, num_cores=NC, trace=False)
        for c, core in enumerate(sim.cores.values()):
            for k, v in in_maps[c].items():
                core.tensor(k)[:] = v
        sim.simulate(check_with_hw=False)
        results = [
            {"out": np.array(core.tensor("out"))}
            for core in sim.cores.values()
        ]
        _last_result["exec_time_ns"] = None
    else:
        nc.finalize()
        br = bass_utils.run_bass_kernel_spmd(
            nc, in_maps, core_ids=list(range(NC)), trace=trace,
        )
        results = br.results
        _last_result["exec_time_ns"] = br.exec_time_ns
        _last_result["profile_json"] = getattr(br, "profile_json", None)

    _last_result["results"] = results
    _last_result["plan"] = plan

    # ---- host unshard ----
    T_tot = plan["T_tot"]
    out = np.empty((N, C), dtype=np.float32)
    for c in range(NC):
        arr = results[c]["out"].reshape(128, T_tot, C).transpose(1, 0, 2)
        arr = arr.reshape(plan["P_OUT"], C)
        rows = plan["out_rows"][c]
        real = rows >= 0
        out[rows[real]] = arr[real]
    return out



# revision 7
# speedup vs baseline: 1.0529x; 1.0529x over previous
"""GCN v2: ap_gather-based edge gather on 8 TRN2 cores.

Layout: nodes sharded 8 ways (core c owns dst range [cP,(c+1)P)).  Tables
live transposed in SBUF as [128 = 8 src-ranges x 16 feats, P nodes]; each
16-partition GPSIMD group gathers edges whose src falls in its range
(ap_gather, group-private int16 index lists).  Per (dst, range) segment
sums run on DVE (exact-K runs, K-desc order, SPMD-global structure);
partials are realigned to global dst order by a second ap_gather and
summed across ranges by one PE matmul.  Self-loop terms are added
directly from the core's own z'/h1' columns (no gather).  Both layers
share one index/schedule set since the graph is identical.
"""

import os
import sys

for _p in ("/opt/trn_rl_repo", "/opt/pypackages"):
    if _p not in sys.path:
        sys.path.insert(0, _p)

import numpy as np

from concourse import bacc, bass, tile, mybir, library_config
from concourse import bass_utils

F32 = mybir.dt.float32
I16 = mybir.dt.int16
AF = mybir.ActivationFunctionType
ALU = mybir.AluOpType

NC = 8
NI = 4096          # gather columns per ap_gather call

_last_result = {}


# ---------------------------------------------------------------------------
# Host-side plan
# ---------------------------------------------------------------------------

def _make_plan(src, dst, N):
    P = N // NC
    core_d = dst // P
    rng_s = src // P
    dloc = dst - core_d * P
    sloc = src - rng_s * P

    # per (core, range): dst counts
    K_cr = []        # K_cr[c][r] = dict-like arrays: (dsts_sorted, counts)
    for c in range(NC):
        row = []
        mc = core_d == c
        for r in range(NC):
            m = mc & (rng_s == r)
            d_ = dloc[m]
            s_ = sloc[m]
            cnt = np.bincount(d_, minlength=P)
            row.append((d_, s_, cnt))
        K_cr.append(row)

    # ONE K-structure shared by all (core, range) pairs so every reduce is
    # full-128-partition (DVE partition offsets must be multiples of 32).
    nd_g = {}
    for r in range(NC):
        for c in range(NC):
            cnt = K_cr[c][r][2]
            ks, nds = np.unique(cnt[cnt > 0], return_counts=True)
            for k, nd in zip(ks, nds):
                nd_g[int(k)] = max(nd_g.get(int(k), 0), int(nd))
    struct = [(k, nd_g[k]) for k in sorted(nd_g, reverse=True)]

    # chunked schedule: entries (coloff, K, nd, ppos); runs never straddle
    # chunk boundaries; identical for every class/core.
    sched = []
    ch = 0
    col = 0
    ppos = 1
    for (k, nd) in struct:
        left = nd
        while left > 0:
            while ch >= len(sched):
                sched.append([])
            fit = min(left, (NI - col) // k)
            if fit == 0:
                ch += 1
                col = 0
                continue
            sched[ch].append((col, k, fit, ppos))
            col += fit * k
            ppos += fit
            left -= fit
    NCH = len(sched)
    SL = NCH * NI
    PW = ppos + 2 - (ppos % 2)  # even pad

    # per-core idx streams + partial position of each (dst, r)
    idx_data = np.zeros((NC, NCH, 128, NI // 16), dtype=np.int16)
    pos_cr = np.full((NC, NC, P), 0, dtype=np.int32)  # [c][r][dst] -> ppos
    for c in range(NC):
        for r in range(NC):
            d_, s_, cnt = K_cr[c][r]
            order = np.lexsort((s_, d_))
            d_s = d_[order]
            s_s = s_[order]
            starts = np.searchsorted(d_s, np.arange(P))
            ends = np.searchsorted(d_s, np.arange(P), side="right")
            # dsts grouped by K desc, dst asc
            ks = cnt.copy()
            # iterate global structure, fill real dsts
            by_k = {}
            for k in sorted(set(ks[ks > 0])):
                by_k[int(k)] = np.where(ks == k)[0]
            used = {int(k): 0 for k in by_k}
            stream = np.zeros(NCH * NI, dtype=np.int16)
            spos = 0  # global stream position (contiguous through chunks)
            # walk the same schedule the device uses
            for ch in range(NCH):
                base = ch * NI
                for (col, k, fit, ppos) in sched[ch]:
                    av = by_k.get(k, np.empty(0, np.int64))
                    u = used.get(k, 0)
                    take = av[u:u + fit]
                    used[k] = u + len(take)
                    for j, dd in enumerate(take):
                        sl = s_s[starts[dd]:ends[dd]]
                        stream[base + col + j * k: base + col + j * k + k] = sl
                        pos_cr[c, r, dd] = ppos + j
            # wrap into tiles: position i -> [16r + i%16, i//16]
            sw = stream.reshape(NCH, NI // 16, 16)
            idx_data[c, :, 16 * r:16 * r + 16, :] = sw.transpose(0, 2, 1)

    # realign indices: rid[c][r][j] = pos_cr or 0, j in [0, 12800)
    NDP = ((P + 511) // 512) * 512  # padded dst cols (512-mult)
    rid_data = np.zeros((NC, 128, NDP // 16), dtype=np.int16)
    for c in range(NC):
        for r in range(NC):
            v = np.zeros(NDP, dtype=np.int16)
            v[:P] = pos_cr[c, r].astype(np.int16)
            rid_data[c, 16 * r:16 * r + 16, :] = v.reshape(NDP // 16, 16).T
    return dict(P=P, SL=SL, NCH=NCH, PW=PW, NDP=NDP, sched=sched,
                idx_data=idx_data, rid_data=rid_data)


# ---------------------------------------------------------------------------
# Device program
# ---------------------------------------------------------------------------

def _ap(t_ap, offset, dims):
    return bass.AP(t_ap.tensor, t_ap.offset + offset, [list(t_ap.ap[0])] + dims)


def _build(N, F, HID, C, plan):
    P = plan["P"]
    NCH = plan["NCH"]
    PW = plan["PW"]
    NDP = plan["NDP"]
    sched = plan["sched"]
    NT2 = NDP // 128          # logits tiles
    KC = F // 128

    nc = bacc.Bacc(None, target_bir_lowering=False, debug=False,
                   num_devices=NC)

    xT_d = nc.dram_tensor("xT", [F, NDP], F32, kind="ExternalInput")
    w1_d = nc.dram_tensor("W1", [F, HID], F32, kind="ExternalInput")
    b1_d = nc.dram_tensor("b1c", [16, 1], F32, kind="ExternalInput")
    w2_d = nc.dram_tensor("W2r", [HID, C], F32, kind="ExternalInput")
    b2_d = nc.dram_tensor("b2r", [128, C], F32, kind="ExternalInput")
    m16_d = nc.dram_tensor("M16", [128, HID], F32, kind="ExternalInput")
    dinv_d = nc.dram_tensor("dinv16", [16, NDP], F32, kind="ExternalInput")
    idx_d = nc.dram_tensor("idxs", [128, NCH * (NI // 16)], I16,
                           kind="ExternalInput")
    rid_d = nc.dram_tensor("rids", [128, NDP // 16], I16,
                           kind="ExternalInput")
    out_d = nc.dram_tensor("out", [128, NT2 * C], F32, kind="ExternalOutput")

    with tile.TileContext(nc) as tc:
        with (
            tc.tile_pool(name="const", bufs=1) as cp,
            tc.tile_pool(name="dram", bufs=1, space="DRAM") as dp,
            tc.tile_pool(name="xt", bufs=3) as xtp,
            tc.tile_pool(name="zp", bufs=2, space="PSUM") as zpp,
            tc.tile_pool(name="zs", bufs=2) as zsp,
            tc.tile_pool(name="tab", bufs=1) as tbp,
            tc.tile_pool(name="idx", bufs=1) as ixp,
            tc.tile_pool(name="g", bufs=2) as gp,
            tc.tile_pool(name="part", bufs=1) as pp,
            tc.tile_pool(name="ra", bufs=2) as rap,
            tc.tile_pool(name="post", bufs=2) as pop,
            tc.tile_pool(name="lp", bufs=2, space="PSUM") as lpp,
        ):
            nc.gpsimd.load_library(library_config.ap_gather)

            w1 = []
            for kc in range(KC):
                t = cp.tile([128, HID], F32, tag=f"w1_{kc}")
                nc.sync.dma_start(out=t[:],
                                  in_=w1_d[kc * 128:(kc + 1) * 128, :])
                w1.append(t)
            b1c = cp.tile([16, 1], F32, tag="b1c")
            nc.sync.dma_start(out=b1c[:], in_=b1_d[:, :])
            w2r = cp.tile([HID, C], F32, tag="w2r")
            nc.sync.dma_start(out=w2r[:], in_=w2_d[:, :])
            b2r = cp.tile([128, C], F32, tag="b2r")
            nc.sync.dma_start(out=b2r[:], in_=b2_d[:, :])
            m16 = cp.tile([128, HID], F32, tag="m16")
            nc.sync.dma_start(out=m16[:], in_=m16_d[:, :])
            idxs = cp.tile([128, NCH * (NI // 16)], I16, tag="idxs")
            nc.sync.dma_start(out=idxs[:], in_=idx_d[:, :])
            rids = cp.tile([128, NDP // 16], I16, tag="rids")
            nc.sync.dma_start(out=rids[:], in_=rid_d[:, :])

            zT_dram = dp.tile([16, NDP], F32, tag="zT")
            h1_dram = dp.tile([16, NDP], F32, tag="h1T")
            zAG_dram = dp.tile([16, P], F32, tag="zAG")
            h1AG_dram = dp.tile([16, P], F32, tag="h1AG")
            tb1_dram = dp.tile([128, P], F32, tag="tb1")
            tb2_dram = dp.tile([128, P], F32, tag="tb2")

            # ---- z'^T = dinv * (x @ W1)^T, in 512-col chunks ----
            for j in range(NDP // 512):
                zp = zpp.tile([16, 512], F32, tag="zp")
                for kc in range(KC):
                    xa = xtp.tile([128, 512], F32, tag="xa")
                    nc.sync.dma_start(
                        out=xa[:],
                        in_=xT_d[kc * 128:(kc + 1) * 128,
                                 j * 512:(j + 1) * 512])
                    nc.tensor.matmul(out=zp[:], lhsT=w1[kc][:], rhs=xa[:],
                                     start=(kc == 0), stop=(kc == KC - 1))
                dv = xtp.tile([16, 512], F32, tag="dv")
                nc.sync.dma_start(out=dv[:],
                                  in_=dinv_d[:, j * 512:(j + 1) * 512])
                zs = zsp.tile([16, 512], F32, tag="zs")
                nc.vector.tensor_tensor(out=zs[:], in0=zp[:], in1=dv[:],
                                        op=ALU.mult)
                nc.sync.dma_start(out=zT_dram[:, j * 512:(j + 1) * 512],
                                  in_=zs[:])

            nc.sync.dma_start(out=zAG_dram[:, :], in_=zT_dram[:, 0:P])
            nc.gpsimd.collective_compute(
                "AllGather", ALU.bypass,
                replica_groups=[list(range(NC))],
                ins=[zAG_dram[:, :]], outs=[tb1_dram[:, :]],
            )

            table = tbp.tile([128, P], F32, tag="table")
            partial = pp.tile([128, PW], F32, tag="partial")

            def layer(table_dram, self_dram, is_last):
                nc.sync.dma_start(out=table[:], in_=table_dram[:, :])
                nc.vector.memset(partial[:], 0.0)
                for ch in range(NCH):
                    gt = gp.tile([128, NI], F32, tag="gt")
                    nc.gpsimd.ap_gather(
                        out_ap=gt[:], in_ap=table[:],
                        idxs_ap=idxs[:, ch * (NI // 16):(ch + 1) * (NI // 16)],
                        channels=128, num_elems=P, d=1, num_idxs=NI,
                    )
                    for (col, k, nd, ppos) in sched[ch]:
                        nc.vector.tensor_reduce(
                            out=partial[:, ppos:ppos + nd],
                            in_=_ap(gt[:], col, [[k, nd], [1, k]]),
                            axis=mybir.AxisListType.X, op=ALU.add,
                        )
                # realign + combine + post, per 512-dst chunk
                RNI = 2048
                nrch = (NDP + RNI - 1) // RNI
                for rc in range(nrch):
                    w = min(RNI, NDP - rc * RNI)
                    ra = rap.tile([128, RNI], F32, tag="ra")
                    nc.gpsimd.ap_gather(
                        out_ap=ra[:, 0:w], in_ap=partial[:],
                        idxs_ap=rids[:, rc * (RNI // 16):
                                     rc * (RNI // 16) + w // 16],
                        channels=128, num_elems=PW, d=1, num_idxs=w,
                    )
                    for j in range(w // 512):
                        cols = slice(rc * RNI + j * 512,
                                     rc * RNI + j * 512 + 512)
                        ap_ = lpp.tile([16, 512], F32, tag="ap_")
                        nc.tensor.matmul(
                            out=ap_[:], lhsT=m16[:],
                            rhs=ra[:, j * 512:(j + 1) * 512],
                            start=True, stop=True)
                        sf = pop.tile([16, 512], F32, tag="sf")
                        nc.sync.dma_start(out=sf[:], in_=self_dram[:, cols])
                        dv = pop.tile([16, 512], F32, tag="dv2")
                        nc.sync.dma_start(out=dv[:], in_=dinv_d[:, cols])
                        ag = pop.tile([16, 512], F32, tag="ag")
                        nc.vector.tensor_tensor(out=ag[:], in0=ap_[:],
                                                in1=sf[:], op=ALU.add)
                        nc.vector.tensor_tensor(out=ag[:], in0=ag[:],
                                                in1=dv[:], op=ALU.mult)
                        if not is_last:
                            nc.vector.tensor_tensor(
                                out=ag[:], in0=ag[:],
                                in1=_ap(b1c[:], 0, [[0, 512]]), op=ALU.add)
                            nc.scalar.activation(out=ag[:], in_=ag[:],
                                                 func=AF.Relu)
                            nc.vector.tensor_tensor(out=ag[:], in0=ag[:],
                                                    in1=dv[:], op=ALU.mult)
                            nc.sync.dma_start(out=h1_dram[:, cols], in_=ag[:])
                        else:
                            # logits + log_softmax per 128-dst tile
                            for i in range(4):
                                t2 = (rc * RNI + j * 512) // 128 + i
                                lp = lpp.tile([128, C], F32, tag="lp")
                                nc.tensor.matmul(
                                    out=lp[:],
                                    lhsT=ag[:, i * 128:(i + 1) * 128],
                                    rhs=w2r[:], start=True, stop=True)
                                lt = pop.tile([128, C], F32, tag="lt")
                                nc.vector.tensor_tensor(
                                    out=lt[:], in0=lp[:], in1=b2r[:],
                                    op=ALU.add)
                                nm = pop.tile([128, 1], F32, tag="nm")
                                nc.vector.tensor_reduce(
                                    out=nm[:], in_=lt[:],
                                    axis=mybir.AxisListType.X,
                                    op=ALU.max, negate=True)
                                nc.vector.tensor_tensor(
                                    out=lt[:], in0=lt[:],
                                    in1=_ap(nm[:], 0, [[0, C]]), op=ALU.add)
                                et = pop.tile([128, C], F32, tag="et")
                                nc.scalar.activation(out=et[:], in_=lt[:],
                                                     func=AF.Exp)
                                nc.vector.tensor_reduce(
                                    out=nm[:], in_=et[:],
                                    axis=mybir.AxisListType.X, op=ALU.add)
                                nc.scalar.activation(out=nm[:], in_=nm[:],
                                                     func=AF.Ln)
                                nc.vector.tensor_tensor(
                                    out=lt[:], in0=lt[:],
                                    in1=_ap(nm[:], 0, [[0, C]]),
                                    op=ALU.subtract)
                                nc.sync.dma_start(
                                    out=out_d[:, t2 * C:(t2 + 1) * C],
                                    in_=lt[:])

            layer(tb1_dram, zT_dram, is_last=False)
            nc.sync.dma_start(out=h1AG_dram[:, :], in_=h1_dram[:, 0:P])
            nc.gpsimd.collective_compute(
                "AllGather", ALU.bypass,
                replica_groups=[list(range(NC))],
                ins=[h1AG_dram[:, :]], outs=[tb2_dram[:, :]],
            )
            layer(tb2_dram, h1_dram, is_last=True)

    return nc


# ---------------------------------------------------------------------------
# Entry point
# ---------------------------------------------------------------------------

def kernel(x, edge_index, W1, b1, W2, b2):
    N, F = x.shape
    HID = W1.shape[1]
    C = W2.shape[1]
    P = N // NC
    src = np.asarray(edge_index[0], dtype=np.int64)
    dst = np.asarray(edge_index[1], dtype=np.int64)

    deg = np.bincount(dst, minlength=N).astype(np.int64) + 1
    dinv = (1.0 / np.sqrt(deg.astype(np.float64))).astype(np.float32)

    plan = _make_plan(src, dst, N)
    NDP = plan["NDP"]
    nc = _build(N, F, HID, C, plan)

    x = np.asarray(x, dtype=np.float32)
    W1 = np.ascontiguousarray(np.asarray(W1, dtype=np.float32))
    W2 = np.ascontiguousarray(np.asarray(W2, dtype=np.float32))
    b2r = np.tile(np.asarray(b2, dtype=np.float32)[None, :], (128, 1))
    M16 = np.zeros((128, HID), dtype=np.float32)
    for r in range(NC):
        M16[16 * r:16 * r + 16, :] = np.eye(HID, dtype=np.float32)

    in_maps = []
    for c in range(NC):
        xT = np.zeros((F, NDP), dtype=np.float32)
        xT[:, :P] = x[c * P:(c + 1) * P].T
        d16 = np.zeros((16, NDP), dtype=np.float32)
        d16[:, :P] = dinv[c * P:(c + 1) * P][None, :]
        in_maps.append({
            "xT": np.ascontiguousarray(xT),
            "W1": W1,
            "b1c": np.ascontiguousarray(
                np.asarray(b1, np.float32).reshape(16, 1)),
            "W2r": W2,
            "b2r": b2r,
            "M16": M16,
            "dinv16": np.ascontiguousarray(d16),
            "idxs": np.ascontiguousarray(
                plan["idx_data"][c].transpose(1, 0, 2).reshape(128, -1)),
            "rids": np.ascontiguousarray(plan["rid_data"][c]),
        })

    trace = bool(int(os.environ.get("GCN_TRACE", "0")))
    if int(os.environ.get("GCN_SIM", "0")):
        from concourse.bass_interp import MultiCoreSim

        sim = MultiCoreSim(nc, num_cores=NC, trace=False)
        for c, core in enumerate(sim.cores.values()):
            for k, v in in_maps[c].items():
                core.tensor(k)[:] = v
        sim.simulate(check_with_hw=False)
        results = [{"out": np.array(core.tensor("out"))}
                   for core in sim.cores.values()]
        _last_result["exec_time_ns"] = None
    else:
        nc.finalize()
        br = bass_utils.run_bass_kernel_spmd(
            nc, in_maps, core_ids=list(range(NC)), trace=trace,
        )
        results = br.results
        _last_result["exec_time_ns"] = br.exec_time_ns

    _last_result["results"] = results
    _last_result["plan"] = plan

    out = np.empty((N, C), dtype=np.float32)
    for c in range(NC):
        arr = results[c]["out"].reshape(128, NDP // 128, C)
        arr = arr.transpose(1, 0, 2).reshape(NDP, C)
        out[c * P:(c + 1) * P] = arr[:P]
    return out
